# revision 1
# baseline (speedup 1.0000x reference)
"""CPDecoding (embedding_lookup) Trainium2 kernel.

out[n] = sum_c fz[c,n]*fy[c,n]*fx[c,n], where f* is a 1-D linear
interpolation (grid_sample, align_corners=True) of a (96, 512) line table
at per-point coordinates.

Strategy (8 cores, data-parallel over the N=4096*192 points):
  - Host: shard points, pre-permute layouts, pack tables as fp16
    [f0 | delta | pad] rows of 512B (one row per table position i holds
    L[:, i] and L[:, i+1]-L[:, i]).
  - Device: per-point (i0, w) on DVE; per-chunk dma_gather of one 512B row
    per (point, axis) from HBM; DVE interp f0 + w*delta, 3-way product,
    free-axis reduce over the 96 components. ~571us/core (cost model),
    memory-bound-adjacent: 151MB of gathered rows per core.
"""

import numpy as np

N_CORES = 8
N_TOTAL = 4096 * 192
N_CORE = N_TOTAL // N_CORES      # 98304 points per core
P = 128                          # partitions
F = N_CORE // P                  # 768 free blocks (wrapped-128 layout)
GROUPS = 8                       # wrapped-16 partition groups
PHI = N_CORE // 16 // GROUPS     # 768 phi-columns per group
C = 96                           # components
R = 512                          # table resolution
ELEM = 256                       # fp16 elements per table row (512 bytes)
CHUNK_F = 32                     # f-blocks per chunk
CHUNK_PTS = P * CHUNK_F          # 4096 points per chunk
N_CHUNKS = F // CHUNK_F          # 24
CHUNKS_PER_GROUP = N_CHUNKS // GROUPS  # 3
SUBCOLS = CHUNK_PTS // 16        # 256 idx columns per chunk

# axis a -> coordinate column in in_tensor (reference samples [z, y, x] from
# pts[:, 2], pts[:, 1], pts[:, 0])
AXIS_COL = [2, 1, 0]

_BUILT = None


def _build_nc():
    """Build the per-core Bass program (SPMD, identical on all cores)."""
    import concourse.bacc as bacc
    import concourse.bass as bass
    import concourse.tile as tile
    from concourse import mybir
    from concourse.library_config import mlp as lib_mlp

    dt = mybir.dt
    Alu = mybir.AluOpType
    Axis = mybir.AxisListType

    nc = bacc.Bacc("TRN2", target_bir_lowering=False, debug=False,
                   num_devices=N_CORES, num_swdge_queues=1)

    pw128 = nc.dram_tensor("pw128", [P, F * 3], dt.float32,
                           kind="ExternalInput").ap()
    pw16 = nc.dram_tensor("pw16", [P, PHI * 3], dt.float32,
                          kind="ExternalInput").ap()
    tbls = [nc.dram_tensor(f"tbl{a}", [R, ELEM], dt.float16,
                           kind="ExternalInput").ap() for a in range(3)]
    out_d = nc.dram_tensor("out", [P, F], dt.float32,
                           kind="ExternalOutput").ap()

    with tile.TileContext(nc) as tc:
        with tc.tile_pool(name="persist", bufs=1) as pp:
            # persistent tiles
            w_axis = [pp.tile([P, F], dt.float16, tag=f"w{a}",
                              name=f"w_axis{a}") for a in range(3)]
            idx_all = pp.tile([P, 3, PHI], dt.int16, tag="idx")
            out_full = pp.tile([P, F], dt.float32, tag="out")

            # ---------- setup: per-point index/weight math ----------
            with tc.tile_pool(name="setup", bufs=1) as sp:
                s128 = sp.tile([P, F * 3], dt.float32, tag="c0")
                nc.sync.dma_start(s128[:], pw128)
                s16 = sp.tile([P, PHI * 3], dt.float32, tag="c1")
                nc.sync.dma_start(s16[:], pw16)

                def idx_chain(src, n_free, want_w, tg):
                    def tmp(nm, dtype=dt.float32):
                        return sp.tile([P, n_free], dtype,
                                       tag="tmp", bufs=4, name=f"{nm}{tg}")
                    # pos = ((coord + 1) * 0.5) * 511, matching jax op order
                    t1 = tmp("t1")
                    nc.vector.tensor_scalar(t1[:], src[:], 1.0, 0.5,
                                            Alu.add, Alu.mult)
                    pos = tmp("pos")
                    nc.vector.tensor_scalar(pos[:], t1[:], 511.0, None,
                                            Alu.mult)
                    # floor(pos) via int round-trip; the fp->int cast may
                    # truncate or round-to-nearest, so fix up with a compare
                    ii = tmp("ii", dt.int32)
                    nc.vector.tensor_copy(ii[:], pos[:])
                    i0f = tmp("i0f")
                    nc.vector.tensor_copy(i0f[:], ii[:])
                    neg = tmp("neg")
                    nc.vector.tensor_tensor(neg[:], pos[:], i0f[:], Alu.is_lt)
                    i0a = tmp("i0a")
                    nc.vector.tensor_sub(i0a[:], i0f[:], neg[:])
                    i0c = tmp("i0c")
                    nc.vector.tensor_scalar(i0c[:], i0a[:], 510.0, 0.0,
                                            Alu.min, Alu.max)
                    if not want_w:
                        return i0c, None
                    w = tmp("w")
                    nc.vector.tensor_sub(w[:], pos[:], i0c[:])
                    return i0c, w

                _, w128 = idx_chain(s128, F * 3, True, "a")
                w128_3d = w128[:].rearrange("p (f k) -> p f k", k=3)
                for a in range(3):
                    nc.vector.tensor_copy(w_axis[a][:],
                                          w128_3d[:, :, AXIS_COL[a]])

                i0c16, _ = idx_chain(s16, PHI * 3, False, "b")
                i16_3d = i0c16[:].rearrange("p (f k) -> p f k", k=3)
                for a in range(3):
                    nc.vector.tensor_copy(idx_all[:, a, :],
                                          i16_3d[:, :, AXIS_COL[a]])

            # ---------- main loop ----------
            with (
                tc.tile_pool(name="stg", bufs=3) as stg_pool,
                tc.tile_pool(name="gath", bufs=2) as gath_pool,
                tc.tile_pool(name="mid", bufs=2) as mid_pool,
            ):
                with tc.tile_critical():
                    nc.gpsimd.load_library(lib_mlp)
                stg_tiles = {}
                for c in range(N_CHUNKS):
                    grp = c // CHUNKS_PER_GROUP
                    sub = c % CHUNKS_PER_GROUP

                    if sub == 0:
                        # stage group grp's indices, replicated into every
                        # 16-partition band (the SWDGE queue's core pair
                        # reads its own 32-partition window)
                        stg = stg_pool.tile([P, 3, PHI], dt.int16, tag="stg")
                        src = idx_all[16 * grp:16 * (grp + 1), :, :]
                        for b in range(8):
                            nc.sync.dma_start(
                                stg[16 * b:16 * (b + 1), :, :], src)
                        stg_tiles[grp] = stg
                    stg = stg_tiles[grp]

                    az = []
                    for a in range(3):
                        g = gath_pool.tile([P, CHUNK_F, ELEM], dt.float16,
                                           tag=f"g{a}")
                        idxs = stg[:, a, SUBCOLS * sub:SUBCOLS * (sub + 1)]
                        nc.gpsimd.dma_gather(
                            g[:], tbls[a], idxs, CHUNK_PTS, CHUNK_PTS, ELEM,
                            elem_step=ELEM, queue_num=0, single_packet=False)

                        f0 = g[:, :, 0:C]
                        dd = g[:, :, C:2 * C]
                        wb = (w_axis[a][:, CHUNK_F * c:CHUNK_F * (c + 1)]
                              .unsqueeze(2).broadcast_to([P, CHUNK_F, C]))
                        u = mid_pool.tile([P, CHUNK_F, C], dt.float16,
                                          tag="u")
                        nc.vector.tensor_mul(u[:], dd, wb)
                        azt = mid_pool.tile([P, CHUNK_F, C], dt.float16,
                                            tag=f"az{a}")
                        nc.vector.tensor_add(azt[:], f0, u[:])
                        az.append(azt)

                    p = mid_pool.tile([P, CHUNK_F, C], dt.float16, tag="p")
                    nc.vector.tensor_mul(p[:], az[0][:], az[1][:])
                    qq = mid_pool.tile([P, CHUNK_F, C], dt.float16, tag="q")
                    nc.vector.tensor_mul(qq[:], p[:], az[2][:])
                    nc.vector.reduce_sum(
                        out_full[:, CHUNK_F * c:CHUNK_F * (c + 1)],
                        qq[:], axis=Axis.X)

                nc.sync.dma_start(out_d, out_full[:])

    nc.compile()
    return nc


def _host_prep(in_tensor, line_z, line_y, line_x):
    """Build per-core input maps (layout permutations + table packing)."""
    pts = np.ascontiguousarray(in_tensor.reshape(-1, 3).astype(np.float32))

    tables = []
    for L in (line_z, line_y, line_x):
        Lf = np.asarray(L, dtype=np.float32)
        f0 = Lf.T                                    # (512, 96)
        f1 = np.concatenate([Lf.T[1:], Lf.T[-1:]], axis=0)
        row = np.zeros((R, ELEM), dtype=np.float16)
        row[:, 0:C] = f0.astype(np.float16)
        row[:, C:2 * C] = (f1 - f0).astype(np.float16)
        tables.append(row)

    in_maps = []
    for k in range(N_CORES):
        shard = pts[k * N_CORE:(k + 1) * N_CORE]
        pw128 = np.ascontiguousarray(
            shard.reshape(F, P, 3).transpose(1, 0, 2).reshape(P, F * 3))
        pw16 = np.ascontiguousarray(
            shard.reshape(GROUPS, PHI, 16, 3).transpose(0, 2, 1, 3)
            .reshape(P, PHI * 3))
        in_maps.append({
            "pw128": pw128,
            "pw16": pw16,
            "tbl0": tables[0],
            "tbl1": tables[1],
            "tbl2": tables[2],
        })
    return in_maps


def _unshard(results):
    outs = []
    for k in range(N_CORES):
        w = np.asarray(results[k]["out"])            # (128, 768), n = 128f + p
        outs.append(w.T.reshape(-1))
    return np.concatenate(outs).reshape(4096, 192).astype(np.float32)


def kernel(in_tensor, line_z, line_y, line_x):
    global _BUILT
    from concourse.bass_utils import run_bass_kernel_spmd

    if _BUILT is None:
        _BUILT = _build_nc()
    nc = _BUILT
    in_maps = _host_prep(np.asarray(in_tensor), np.asarray(line_z),
                         np.asarray(line_y), np.asarray(line_x))
    res = run_bass_kernel_spmd(nc, in_maps, list(range(N_CORES)))
    return _unshard(res.results)



# revision 3
# speedup vs baseline: 1.5775x; 1.5775x over previous
"""CPDecoding (embedding_lookup) Trainium2 kernel.

out[n] = sum_c fz[c,n]*fy[c,n]*fx[c,n], where f* is a 1-D linear
interpolation (grid_sample, align_corners=True) of a (96, 512) line table
at per-point coordinates.

Strategy (8 cores, data-parallel over the N=4096*192 points):
  - Host: shard points; pre-upsample each line table 64x (linear interp
    evaluated on a 32705-point grid, fp16, rows padded to 256B) so the
    device needs NO interpolation FMA -- one 256B row gather per
    (point, axis) via SWDGE dma_gather with int16 indices.
  - Device: per-point q = round(pos*64) on DVE; per-chunk dma_gather of
    one row per (point, axis); fp16 products + tree reduction over the
    96 components.
"""

import numpy as np

N_CORES = 8
N_TOTAL = 4096 * 192
N_CORE = N_TOTAL // N_CORES      # 98304 points per core
P = 128                          # partitions
F = N_CORE // P                  # 768 f-cols (pt-layout: n = 128*f + p)
GROUPS = 8                       # wrapped-16 partition groups
PHI = N_CORE // 16 // GROUPS     # 768 idx-cols per group
C = 96                           # components
R = 512                          # original table resolution
S = 64                           # upsample factor
NQ = (R - 1) * S + 1             # 32705 upsampled rows (int16-indexable)
ELEM = 128                       # fp16 elements per table row (256 bytes)
CHUNK_F = 32                     # f-cols per chunk
CHUNK_PTS = P * CHUNK_F          # 4096 points per chunk
N_CHUNKS = F // CHUNK_F          # 24
CHUNKS_PER_GROUP = N_CHUNKS // GROUPS  # 3
SUBCOLS = CHUNK_PTS // 16        # 256 idx columns per chunk
QSCALE = (R - 1) * S             # pos*S = t1*QSCALE for t1 = (coord+1)/2

# axis a -> coordinate column in in_tensor (reference samples [z, y, x] from
# pts[:, 2], pts[:, 1], pts[:, 0])
AXIS_COL = [2, 1, 0]

_BUILT = None


def _build_nc():
    """Build the per-core Bass program (SPMD, identical on all cores)."""
    import concourse.bacc as bacc
    import concourse.bass as bass
    import concourse.tile as tile
    from concourse import mybir
    from concourse.library_config import mlp as lib_mlp

    dt = mybir.dt
    Alu = mybir.AluOpType
    Axis = mybir.AxisListType

    nc = bacc.Bacc("TRN2", target_bir_lowering=False, debug=False,
                   num_devices=N_CORES, num_swdge_queues=1)

    pw16 = nc.dram_tensor("pw16", [P, PHI * 3], dt.float32,
                          kind="ExternalInput").ap()
    tbls = [nc.dram_tensor(f"tbl{a}", [NQ, ELEM], dt.float16,
                           kind="ExternalInput").ap() for a in range(3)]
    out_d = nc.dram_tensor("out", [P, F], dt.float32,
                           kind="ExternalOutput").ap()

    with tile.TileContext(nc) as tc:
        with tc.tile_pool(name="persist", bufs=1) as pp:
            idx_all = pp.tile([P, 3, PHI], dt.int16, tag="idx")
            out_full = pp.tile([P, F], dt.float32, tag="out")

            # ---------- setup: per-point index math ----------
            with tc.tile_pool(name="setup", bufs=1) as sp:
                s16 = sp.tile([P, PHI * 3], dt.float32, tag="c1")
                nc.sync.dma_start(s16[:], pw16)

                def tmp(nm, dtype=dt.float32):
                    return sp.tile([P, PHI * 3], dtype,
                                   tag="tmp", bufs=4, name=nm)
                # t1 = (coord + 1) * 0.5, matching jax op order, then
                # q = clamp(floor(t1 * QSCALE + 0.5), 0, NQ - 1)
                t1 = tmp("t1")
                nc.vector.tensor_scalar(t1[:], s16[:], 1.0, 0.5,
                                        Alu.add, Alu.mult)
                pos = tmp("pos")
                nc.vector.tensor_scalar(pos[:], t1[:], float(QSCALE), 0.5,
                                        Alu.mult, Alu.add)
                # floor(pos) via int round-trip; the fp->int cast may
                # truncate or round-to-nearest, so fix up with a compare
                ii = tmp("ii", dt.int32)
                nc.vector.tensor_copy(ii[:], pos[:])
                i0f = tmp("i0f")
                nc.vector.tensor_copy(i0f[:], ii[:])
                neg = tmp("neg")
                nc.vector.tensor_tensor(neg[:], pos[:], i0f[:], Alu.is_lt)
                i0a = tmp("i0a")
                nc.vector.tensor_sub(i0a[:], i0f[:], neg[:])
                i0c = tmp("i0c")
                nc.vector.tensor_scalar(i0c[:], i0a[:], float(NQ - 1), 0.0,
                                        Alu.min, Alu.max)
                i16_3d = i0c[:].rearrange("p (f k) -> p f k", k=3)
                for a in range(3):
                    nc.vector.tensor_copy(idx_all[:, a, :],
                                          i16_3d[:, :, AXIS_COL[a]])

            # ---------- main loop ----------
            with (
                tc.tile_pool(name="stg", bufs=3) as stg_pool,
                tc.tile_pool(name="gath", bufs=2) as gath_pool,
                tc.tile_pool(name="mid", bufs=2) as mid_pool,
            ):
                with tc.tile_critical():
                    nc.gpsimd.load_library(lib_mlp)
                stg_tiles = {}
                for c in range(N_CHUNKS):
                    grp = c // CHUNKS_PER_GROUP
                    sub = c % CHUNKS_PER_GROUP

                    if sub == 0:
                        # stage group grp's indices, replicated into every
                        # 16-partition band (the SWDGE queue's core pair
                        # reads its own 32-partition window)
                        stg = stg_pool.tile([P, 3, PHI], dt.int16, tag="stg")
                        src = idx_all[16 * grp:16 * (grp + 1), :, :]
                        for b in range(8):
                            nc.sync.dma_start(
                                stg[16 * b:16 * (b + 1), :, :], src)
                        stg_tiles[grp] = stg
                    stg = stg_tiles[grp]

                    az = []
                    for a in range(3):
                        g = gath_pool.tile([P, CHUNK_F, ELEM], dt.float16,
                                           tag=f"g{a}")
                        idxs = stg[:, a, SUBCOLS * sub:SUBCOLS * (sub + 1)]
                        nc.gpsimd.dma_gather(
                            g[:], tbls[a], idxs, CHUNK_PTS, CHUNK_PTS, ELEM,
                            elem_step=ELEM, queue_num=0, single_packet=False)
                        az.append(g)

                    p1 = mid_pool.tile([P, CHUNK_F, C], dt.float16, tag="p1")
                    nc.vector.tensor_mul(p1[:], az[0][:, :, 0:C],
                                         az[1][:, :, 0:C])
                    p2 = mid_pool.tile([P, CHUNK_F, C], dt.float16, tag="p2")
                    nc.vector.tensor_mul(p2[:], p1[:], az[2][:, :, 0:C])
                    cur, width = p2, C
                    while width > 6:
                        h = width // 2
                        nxt = mid_pool.tile([P, CHUNK_F, h], dt.float16,
                                            tag=f"t{width}")
                        nc.vector.tensor_add(nxt[:], cur[:, :, 0:h],
                                             cur[:, :, h:width])
                        cur, width = nxt, h
                    nc.vector.reduce_sum(
                        out_full[:, CHUNK_F * c:CHUNK_F * (c + 1)],
                        cur[:], axis=Axis.X)

                nc.sync.dma_start(out_d, out_full[:])

    nc.compile()
    return nc


def _build_tables(line_z, line_y, line_x):
    """Upsample each (C, R) table to (NQ, ELEM) fp16 rows (linear interp)."""
    qs = np.arange(NQ)
    i0 = np.minimum(qs // S, R - 2)
    w = (qs / S - i0).astype(np.float32)[None, :]
    out = []
    for L in (line_z, line_y, line_x):
        Lf = np.asarray(L, dtype=np.float32)
        Uq = Lf[:, i0] * (1.0 - w) + Lf[:, i0 + 1] * w       # [C, NQ]
        row = np.zeros((NQ, ELEM), dtype=np.float16)
        row[:, 0:C] = Uq.T.astype(np.float16)
        out.append(row)
    return out


def _host_prep(in_tensor, line_z, line_y, line_x):
    """Build per-core input maps (layout permutation + table upsampling)."""
    pts = np.ascontiguousarray(in_tensor.reshape(-1, 3).astype(np.float32))
    tables = _build_tables(line_z, line_y, line_x)

    in_maps = []
    for k in range(N_CORES):
        shard = pts[k * N_CORE:(k + 1) * N_CORE]
        pw16 = np.ascontiguousarray(
            shard.reshape(GROUPS, PHI, 16, 3).transpose(0, 2, 1, 3)
            .reshape(P, PHI * 3))
        in_maps.append({
            "pw16": pw16,
            "tbl0": tables[0],
            "tbl1": tables[1],
            "tbl2": tables[2],
        })
    return in_maps


def _unshard(results):
    outs = []
    for k in range(N_CORES):
        w = np.asarray(results[k]["out"])            # (128, 768), n = 128f + p
        outs.append(w.T.reshape(-1))
    return np.concatenate(outs).reshape(4096, 192).astype(np.float32)


def kernel(in_tensor, line_z, line_y, line_x):
    global _BUILT
    from concourse.bass_utils import run_bass_kernel_spmd

    if _BUILT is None:
        _BUILT = _build_nc()
    nc = _BUILT
    in_maps = _host_prep(np.asarray(in_tensor), np.asarray(line_z),
                         np.asarray(line_y), np.asarray(line_x))
    res = run_bass_kernel_spmd(nc, in_maps, list(range(N_CORES)))
    return _unshard(res.results)
